# revision 26
# baseline (speedup 1.0000x reference)
import sys

import numpy as np

for p in ("/opt/trn_rl_repo",):
    if p not in sys.path:
        sys.path.insert(0, p)

import ml_dtypes  # noqa: E402

import concourse.tile as tile  # noqa: E402
from concourse import bacc, mybir  # noqa: E402
from concourse.bass_utils import run_bass_kernel_spmd  # noqa: E402

B, N, D = 128, 512, 512
NCORES = 8
BPC = B // NCORES  # 16 batch items per core
F32 = mybir.dt.float32
BF16 = mybir.dt.bfloat16
ACT_COPY = mybir.ActivationFunctionType.Copy


def _hadamard(n: int) -> np.ndarray:
    H = np.array([[1.0]], dtype=np.float32)
    base = np.array([[1.0, 1.0], [1.0, -1.0]], dtype=np.float32)
    while H.shape[0] < n:
        H = np.kron(H, base)
    return H


def _build():
    # y = H512 @ x @ H512 / 512 per item, bf16 device I/O.  DMA floor =
    # 16.8MB/core at 360B/ns = 46.6us; engines are balanced to ~3us/item.
    #
    # H512 = (H2 (x) I256)(I2 (x) H2 (x) I128)(I4 (x) H128); all factors
    # commute, so butterfly levels run pre-matmul on cheap bf16 SBUF ops:
    #   F1 row level (nblk pairs (0,2),(1,3))     - DVE
    #   G1 col level (d halves)                   - add DVE, sub Pool
    #   F2 row level (pairs (0,1),(2,3)): block 0 on DVE; blocks 1,2,3
    #     absorbed into K=256 left matmuls (2 PSUM-accum steps, H256 halves
    #     or +-H128) - trades cheap PE rows for scarce DVE throughput.
    # left PE  -> t^T PSUM [d-chunk, n] (4 banks)
    # middle eviction: Act 2 banks + DVE 2 banks -> tt bf16
    # right PE (K=256 vs H256/512 halves) -> y natural [n-chunk, e] (4 banks)
    # final eviction: Act one 4-bank op -> yt bf16 -> DMA out.
    #
    # Software pipeline per iteration k: load(k+3) | pre(k+1) | left(k+1)
    # trails pre by one | mid(k) | right(k-1), fin(k-1), store(k-1) - PE
    # alternates left/right of different items so it never waits on an
    # eviction; PSUM is exactly tp(4)+yp(4) banks with bufs=1 each.
    nc = bacc.Bacc("TRN2", target_bir_lowering=False, debug=False)
    x_d = nc.dram_tensor("x", [BPC, 4, 128, D], BF16, kind="ExternalInput").ap()
    hc_d = nc.dram_tensor("hc", [128, 2, 2, 256], BF16, kind="ExternalInput").ap()
    y_d = nc.dram_tensor("y", [BPC, 4, 128, D], BF16, kind="ExternalOutput").ap()

    with tile.TileContext(nc) as tc:
        with (
            tc.tile_pool(name="const", bufs=1) as cpool,
            tc.tile_pool(name="xp", bufs=6) as xpool,
            tc.tile_pool(name="xa", bufs=6) as apool,
            tc.tile_pool(name="xb", bufs=6) as bpool,
            tc.tile_pool(name="xc", bufs=6) as ccpool,
            tc.tile_pool(name="tt", bufs=4) as ttpool,
            tc.tile_pool(name="yt", bufs=4) as ytpool,
            tc.tile_pool(name="tp", bufs=2, space="PSUM") as tppool,
            tc.tile_pool(name="yp", bufs=2, space="PSUM") as yppool,
        ):
            hc = cpool.tile([128, 2, 2, 256], BF16)
            h256r = hc[:, 0]   # [128, 2, 256] rows of H256, split in halves
            hs256r = hc[:, 1]  # H256 / 512
            h128 = hc[:, 0, 0, 0:128]        # H256 = [[H,H],[H,-H]]
            h128n = hc[:, 0, 1, 128:256]     # -H128

            # Warm the activation table at t=0 (1283ns load otherwise lands
            # on the first mid-eviction, stretching the pipeline head).
            warm = cpool.tile([128, 1], BF16)
            nc.vector.memset(warm[:], 0.0)
            nc.scalar.activation(warm[:], warm[:], ACT_COPY)

            def stage_load(b):
                xt = xpool.tile([128, 4, D], BF16, tag="xt", name="xt")
                xsrc = x_d[b].transpose([1, 0, 2])
                if b == 0:
                    # Head: tiny constants DMA first, then x0 in d-halves so
                    # F1 starts while the high half is still in flight.
                    nc.sync.dma_start(hc[:], hc_d[:])
                    nc.sync.dma_start(xt[:, :, 0:256], xsrc[:, :, 0:256])
                    nc.sync.dma_start(xt[:, :, 256:512], xsrc[:, :, 256:512])
                else:
                    nc.sync.dma_start(xt[:], xsrc)
                return xt

            def stage_pre_a(xt, split_f1=False, dve_all=False):
                # F1 row level: pairs (0,2), (1,3)
                xa = apool.tile([128, 4, D], BF16, tag="xa", name="xa")
                if split_f1:
                    # Item 0: split F1 by d-halves to chase the two load
                    # pieces -- the pre chain is the pipeline-head critical
                    # path.
                    for sl in (slice(0, 256), slice(256, 512)):
                        nc.vector.tensor_add(
                            xa[:, 0:2, sl], xt[:, 0:2, sl], xt[:, 2:4, sl]
                        )
                        nc.vector.tensor_sub(
                            xa[:, 2:4, sl], xt[:, 0:2, sl], xt[:, 2:4, sl]
                        )
                else:
                    nc.vector.tensor_add(xa[:, 0:2], xt[:, 0:2], xt[:, 2:4])
                    nc.vector.tensor_sub(xa[:, 2:4], xt[:, 0:2], xt[:, 2:4])
                # G1 col level: d halves (sub on Pool to offload DVE in
                # steady state; head items keep it on the then-idle DVE)
                xb = bpool.tile([128, 4, D], BF16, tag="xb", name="xb")
                nc.vector.tensor_add(
                    xb[:, :, 0:256], xa[:, :, 0:256], xa[:, :, 256:512]
                )
                sub_eng = nc.vector if dve_all else nc.gpsimd
                sub_eng.tensor_sub(
                    xb[:, :, 256:512], xa[:, :, 0:256], xa[:, :, 256:512]
                )
                return xb

            def stage_pre_b(xb, dve=False):
                # F2 row level, output block 0 only (= xb0 + xb1); blocks
                # 1,2,3 are absorbed into the K=256 left matmuls.
                xc = ccpool.tile([128, 1, D], BF16, tag="xc", name="xc")
                eng = nc.vector if dve else nc.gpsimd
                eng.tensor_add(xc[:, 0], xb[:, 0], xb[:, 1])
                return xc

            def _left_mms(tp, xb, xc, j):
                dsl = slice(128 * j, 128 * (j + 1))
                jj = j % 2
                # n-block 0: F2 pre-applied, K=128
                nc.tensor.matmul(
                    tp[:, jj, 0:128], xc[:, 0, dsl], h128,
                    start=True, stop=True,
                )
                # n-block 1 = xb0 - xb1: K=256 via +-H128
                nc.tensor.matmul(
                    tp[:, jj, 128:256], xb[:, 0, dsl], h128,
                    start=True, stop=False,
                )
                nc.tensor.matmul(
                    tp[:, jj, 128:256], xb[:, 1, dsl], h128n,
                    start=False, stop=True,
                )
                # n-blocks 2,3: K=256 vs H256 row-halves
                for s in range(2):
                    nc.tensor.matmul(
                        tp[:, jj, 256:512], xb[:, 2 + s, dsl], h256r[:, s],
                        start=(s == 0), stop=(s == 1),
                    )

            def stage_left_mid(xb, xc):
                # t^T in two 2-bank PSUM tiles; each is evicted as soon as
                # its half of the matmuls lands, so the next item's left
                # matmuls only wait on the matching half-eviction (bufs=2).
                tt = ttpool.tile([128, 4, D], BF16, tag="tt", name="tt")
                tpa = tppool.tile([128, 2, D], F32, tag="tp", name="tpa")
                for j in (0, 1):
                    _left_mms(tpa, xb, xc, j)
                nc.scalar.activation(tt[:, 0:2], tpa[:], ACT_COPY)
                tpb = tppool.tile([128, 2, D], F32, tag="tp", name="tpb")
                for j in (2, 3):
                    _left_mms(tpb, xb, xc, j)
                nc.vector.tensor_copy(tt[:, 2:4], tpb[:])
                return tt

            def _right_mms(yp, tt, c):
                nsl = slice(128 * c, 128 * (c + 1))
                for h in range(2):
                    for s in range(2):
                        nc.tensor.matmul(
                            yp[:, c % 2, 256 * h:256 * (h + 1)],
                            tt[:, 2 * h + s, nsl],
                            hs256r[:, s],
                            start=(s == 0), stop=(s == 1),
                        )

            def stage_right_out(b, tt, fine=0):
                # y in two 2-bank PSUM tiles with per-half eviction (Act).
                # fine=1 (next-to-last item): store goes out in halves.
                # fine=2 (last item): also per-bank Act/DVE evictions.
                yt = ytpool.tile([128, 4, D], BF16, tag="yt", name="yt")
                ydst = y_d[b].transpose([1, 0, 2])
                ypa = yppool.tile([128, 2, D], F32, tag="yp", name="ypa")
                for c in (0, 1):
                    _right_mms(ypa, tt, c)
                if fine == 2:
                    nc.scalar.activation(yt[:, 0], ypa[:, 0], ACT_COPY)
                    nc.vector.tensor_copy(yt[:, 1], ypa[:, 1])
                else:
                    nc.scalar.activation(yt[:, 0:2], ypa[:], ACT_COPY)
                if fine:
                    nc.sync.dma_start(ydst[:, 0:2], yt[:, 0:2])
                ypb = yppool.tile([128, 2, D], F32, tag="yp", name="ypb")
                for c in (2, 3):
                    _right_mms(ypb, tt, c)
                if fine == 2:
                    nc.scalar.activation(yt[:, 2], ypb[:, 0], ACT_COPY)
                    nc.vector.tensor_copy(yt[:, 3], ypb[:, 1])
                else:
                    nc.scalar.activation(yt[:, 2:4], ypb[:], ACT_COPY)
                if fine:
                    nc.sync.dma_start(ydst[:, 2:4], yt[:, 2:4])
                else:
                    nc.sync.dma_start(ydst, yt[:])

            # prologue
            xts, xbs, xcs, tts = {}, {}, {}, {}
            for b in range(min(3, BPC)):
                xts[b] = _logged(nc, f"load({b})", stage_load, b)
            xbs[0] = _logged(nc, "pre_a(0)", stage_pre_a, xts.pop(0),
                             True, True)
            xcs[0] = _logged(nc, "pre_b(0)", stage_pre_b, xbs[0], True)
            # steady state: iteration k handles pre_a(k+1), left(k), mid(k),
            # pre_b(k+1), right(k-1), fin(k-1), store(k-1), load(k+3)
            for k in range(BPC + 1):
                if k < BPC:
                    # pre_a(k+1) first: its DVE ops precede mid(k)'s copy in
                    # the in-order DVE queue, so DVE works on pre(k+1) while
                    # PE runs left(k) instead of stalling on mid(k)'s dep.
                    # pre_b(k+1) (waits on Pool's G1-sub) comes after mid(k).
                    if k + 1 < BPC:
                        head = k + 1 <= 2  # items 1,2: all-DVE pre levels
                        xbs[k + 1] = _logged(
                            nc, f"pre_a({k + 1})", stage_pre_a,
                            xts.pop(k + 1), False, head)
                    tts[k] = _logged(nc, f"left({k})", stage_left_mid,
                                     xbs.pop(k), xcs.pop(k))
                    if k + 1 < BPC:
                        xcs[k + 1] = _logged(
                            nc, f"pre_b({k + 1})", stage_pre_b,
                            xbs[k + 1], k + 1 <= 2)
                    if k + 3 < BPC:
                        xts[k + 3] = _logged(
                            nc, f"load({k + 3})", stage_load, k + 3)
                if k - 1 >= 0:
                    fine = max(0, (k - 1) - (BPC - 3))
                    _logged(nc, f"right({k - 1})", stage_right_out,
                            k - 1, tts.pop(k - 1), fine)

    nc.compile()
    return nc


_NC = None
STAGE_LOG = []  # (stage_label, first_inst_idx, last_inst_idx) debug aid


def _logged(nc, label, fn, *args):
    n0 = int(nc.get_next_instruction_name().split("-")[1])
    out = fn(*args)
    n1 = int(nc.get_next_instruction_name().split("-")[1])
    STAGE_LOG.append((label, n0, n1))
    return out


def kernel(x: np.ndarray) -> np.ndarray:
    global _NC
    if _NC is None:
        _NC = _build()
    x = np.ascontiguousarray(
        np.asarray(x, dtype=np.float32).astype(ml_dtypes.bfloat16)
    )
    H = _hadamard(256)
    # hc[p, 0, s, q] = H256[s*128+p, q]; hc[p, 1, s, q] = same / 512
    hrows = H.reshape(2, 128, 256).transpose(1, 0, 2)  # [128, 2, 256]
    hc = np.stack([hrows, hrows / np.float32(512.0)], axis=1)
    hc = np.ascontiguousarray(hc.astype(ml_dtypes.bfloat16))
    xr = x.reshape(NCORES, BPC, 4, 128, D)
    in_maps = [{"x": xr[i], "hc": hc} for i in range(NCORES)]
    res = run_bass_kernel_spmd(_NC, in_maps, list(range(NCORES))).results
    return np.concatenate(
        [np.asarray(r["y"]).reshape(BPC, N, D) for r in res], axis=0
    ).astype(np.float32)


# revision 28
# speedup vs baseline: 1.0542x; 1.0542x over previous
import sys

import numpy as np

for p in ("/opt/trn_rl_repo",):
    if p not in sys.path:
        sys.path.insert(0, p)

import ml_dtypes  # noqa: E402

import concourse.tile as tile  # noqa: E402
from concourse import bacc, mybir  # noqa: E402
from concourse.bass_utils import run_bass_kernel_spmd  # noqa: E402

B, N, D = 128, 512, 512
NCORES = 8
BPC = B // NCORES  # 16 batch items per core
HEAD_ITEMS = 0  # items <= this get all-DVE pre levels (pipeline head)
FINE_TAIL = 2   # how many trailing items get fine-grained drain
F32 = mybir.dt.float32
BF16 = mybir.dt.bfloat16
ACT_COPY = mybir.ActivationFunctionType.Copy


def _hadamard(n: int) -> np.ndarray:
    H = np.array([[1.0]], dtype=np.float32)
    base = np.array([[1.0, 1.0], [1.0, -1.0]], dtype=np.float32)
    while H.shape[0] < n:
        H = np.kron(H, base)
    return H


def _build():
    # y = H512 @ x @ H512 / 512 per item, bf16 device I/O.  DMA floor =
    # 16.8MB/core at 360B/ns = 46.6us; engines are balanced to ~3us/item.
    #
    # H512 = (H2 (x) I256)(I2 (x) H2 (x) I128)(I4 (x) H128); all factors
    # commute, so butterfly levels run pre-matmul on cheap bf16 SBUF ops:
    #   F1 row level (nblk pairs (0,2),(1,3))     - DVE
    #   G1 col level (d halves)                   - add DVE, sub Pool
    #   F2 row level (pairs (0,1),(2,3)): block 0 on DVE; blocks 1,2,3
    #     absorbed into K=256 left matmuls (2 PSUM-accum steps, H256 halves
    #     or +-H128) - trades cheap PE rows for scarce DVE throughput.
    # left PE  -> t^T PSUM [d-chunk, n] (4 banks)
    # middle eviction: Act 2 banks + DVE 2 banks -> tt bf16
    # right PE (K=256 vs H256/512 halves) -> y natural [n-chunk, e] (4 banks)
    # final eviction: Act one 4-bank op -> yt bf16 -> DMA out.
    #
    # Software pipeline per iteration k: load(k+3) | pre(k+1) | left(k+1)
    # trails pre by one | mid(k) | right(k-1), fin(k-1), store(k-1) - PE
    # alternates left/right of different items so it never waits on an
    # eviction; PSUM is exactly tp(4)+yp(4) banks with bufs=1 each.
    nc = bacc.Bacc("TRN2", target_bir_lowering=False, debug=False)
    x_d = nc.dram_tensor("x", [BPC, 4, 128, D], BF16, kind="ExternalInput").ap()
    hc_d = nc.dram_tensor("hc", [128, 2, 2, 256], BF16, kind="ExternalInput").ap()
    y_d = nc.dram_tensor("y", [BPC, 4, 128, D], BF16, kind="ExternalOutput").ap()

    with tile.TileContext(nc) as tc:
        with (
            tc.tile_pool(name="const", bufs=1) as cpool,
            tc.tile_pool(name="xp", bufs=6) as xpool,
            tc.tile_pool(name="xa", bufs=6) as apool,
            tc.tile_pool(name="xb", bufs=6) as bpool,
            tc.tile_pool(name="xc", bufs=6) as ccpool,
            tc.tile_pool(name="tt", bufs=4) as ttpool,
            tc.tile_pool(name="yt", bufs=4) as ytpool,
            tc.tile_pool(name="tp", bufs=2, space="PSUM") as tppool,
            tc.tile_pool(name="yp", bufs=2, space="PSUM") as yppool,
        ):
            hc = cpool.tile([128, 2, 2, 256], BF16)
            h256r = hc[:, 0]   # [128, 2, 256] rows of H256, split in halves
            hs256r = hc[:, 1]  # H256 / 512
            h128 = hc[:, 0, 0, 0:128]        # H256 = [[H,H],[H,-H]]
            h128n = hc[:, 0, 1, 128:256]     # -H128

            # Warm the activation table at t=0 (1283ns load otherwise lands
            # on the first mid-eviction, stretching the pipeline head).
            warm = cpool.tile([128, 1], BF16)
            nc.vector.memset(warm[:], 0.0)
            nc.scalar.activation(warm[:], warm[:], ACT_COPY)

            def stage_load(b):
                xt = xpool.tile([128, 4, D], BF16, tag="xt", name="xt")
                xsrc = x_d[b].transpose([1, 0, 2])
                if b == 0:
                    # Head: tiny constants DMA first, then x0 in d-halves so
                    # F1 starts while the high half is still in flight.
                    nc.sync.dma_start(hc[:], hc_d[:])
                    nc.sync.dma_start(xt[:, :, 0:256], xsrc[:, :, 0:256])
                    nc.sync.dma_start(xt[:, :, 256:512], xsrc[:, :, 256:512])
                else:
                    nc.sync.dma_start(xt[:], xsrc)
                return xt

            def stage_pre_a(xt, split_f1=False, dve_all=False):
                # F1 row level: pairs (0,2), (1,3)
                xa = apool.tile([128, 4, D], BF16, tag="xa", name="xa")
                if split_f1:
                    # Item 0: split F1 by d-halves to chase the two load
                    # pieces -- the pre chain is the pipeline-head critical
                    # path.
                    for sl in (slice(0, 256), slice(256, 512)):
                        nc.vector.tensor_add(
                            xa[:, 0:2, sl], xt[:, 0:2, sl], xt[:, 2:4, sl]
                        )
                        nc.vector.tensor_sub(
                            xa[:, 2:4, sl], xt[:, 0:2, sl], xt[:, 2:4, sl]
                        )
                else:
                    nc.vector.tensor_add(xa[:, 0:2], xt[:, 0:2], xt[:, 2:4])
                    nc.vector.tensor_sub(xa[:, 2:4], xt[:, 0:2], xt[:, 2:4])
                # G1 col level: d halves (sub on Pool to offload DVE in
                # steady state; head items keep it on the then-idle DVE)
                xb = bpool.tile([128, 4, D], BF16, tag="xb", name="xb")
                nc.vector.tensor_add(
                    xb[:, :, 0:256], xa[:, :, 0:256], xa[:, :, 256:512]
                )
                sub_eng = nc.vector if dve_all else nc.gpsimd
                sub_eng.tensor_sub(
                    xb[:, :, 256:512], xa[:, :, 0:256], xa[:, :, 256:512]
                )
                return xb

            def stage_pre_b(xb, dve=False):
                # F2 row level, output block 0 only (= xb0 + xb1); blocks
                # 1,2,3 are absorbed into the K=256 left matmuls.
                xc = ccpool.tile([128, 1, D], BF16, tag="xc", name="xc")
                eng = nc.vector if dve else nc.gpsimd
                eng.tensor_add(xc[:, 0], xb[:, 0], xb[:, 1])
                return xc

            def _left_mms(tp, xb, xc, j):
                dsl = slice(128 * j, 128 * (j + 1))
                jj = j % 2
                # n-block 0: F2 pre-applied, K=128
                nc.tensor.matmul(
                    tp[:, jj, 0:128], xc[:, 0, dsl], h128,
                    start=True, stop=True,
                )
                # n-block 1 = xb0 - xb1: K=256 via +-H128
                nc.tensor.matmul(
                    tp[:, jj, 128:256], xb[:, 0, dsl], h128,
                    start=True, stop=False,
                )
                nc.tensor.matmul(
                    tp[:, jj, 128:256], xb[:, 1, dsl], h128n,
                    start=False, stop=True,
                )
                # n-blocks 2,3: K=256 vs H256 row-halves
                for s in range(2):
                    nc.tensor.matmul(
                        tp[:, jj, 256:512], xb[:, 2 + s, dsl], h256r[:, s],
                        start=(s == 0), stop=(s == 1),
                    )

            def stage_left_mid(xb, xc):
                # t^T in two 2-bank PSUM tiles; each is evicted as soon as
                # its half of the matmuls lands, so the next item's left
                # matmuls only wait on the matching half-eviction (bufs=2).
                tt = ttpool.tile([128, 4, D], BF16, tag="tt", name="tt")
                tpa = tppool.tile([128, 2, D], F32, tag="tp", name="tpa")
                for j in (0, 1):
                    _left_mms(tpa, xb, xc, j)
                nc.scalar.activation(tt[:, 0:2], tpa[:], ACT_COPY)
                tpb = tppool.tile([128, 2, D], F32, tag="tp", name="tpb")
                for j in (2, 3):
                    _left_mms(tpb, xb, xc, j)
                nc.vector.tensor_copy(tt[:, 2:4], tpb[:])
                return tt

            def _right_mms(yp, tt, c):
                nsl = slice(128 * c, 128 * (c + 1))
                for h in range(2):
                    for s in range(2):
                        nc.tensor.matmul(
                            yp[:, c % 2, 256 * h:256 * (h + 1)],
                            tt[:, 2 * h + s, nsl],
                            hs256r[:, s],
                            start=(s == 0), stop=(s == 1),
                        )

            def stage_right_out(b, tt, fine=0):
                # y in two 2-bank PSUM tiles with per-half eviction (Act).
                # fine=1 (next-to-last item): store goes out in halves.
                # fine=2 (last item): also per-bank Act/DVE evictions.
                yt = ytpool.tile([128, 4, D], BF16, tag="yt", name="yt")
                ydst = y_d[b].transpose([1, 0, 2])
                ypa = yppool.tile([128, 2, D], F32, tag="yp", name="ypa")
                for c in (0, 1):
                    _right_mms(ypa, tt, c)
                if fine == 2:
                    nc.scalar.activation(yt[:, 0], ypa[:, 0], ACT_COPY)
                    nc.vector.tensor_copy(yt[:, 1], ypa[:, 1])
                else:
                    nc.scalar.activation(yt[:, 0:2], ypa[:], ACT_COPY)
                if fine:
                    nc.sync.dma_start(ydst[:, 0:2], yt[:, 0:2])
                ypb = yppool.tile([128, 2, D], F32, tag="yp", name="ypb")
                for c in (2, 3):
                    _right_mms(ypb, tt, c)
                if fine == 2:
                    nc.scalar.activation(yt[:, 2], ypb[:, 0], ACT_COPY)
                    nc.vector.tensor_copy(yt[:, 3], ypb[:, 1])
                else:
                    nc.scalar.activation(yt[:, 2:4], ypb[:], ACT_COPY)
                if fine:
                    nc.sync.dma_start(ydst[:, 2:4], yt[:, 2:4])
                else:
                    nc.sync.dma_start(ydst, yt[:])

            # prologue
            xts, xbs, xcs, tts = {}, {}, {}, {}
            for b in range(min(3, BPC)):
                xts[b] = _logged(nc, f"load({b})", stage_load, b)
            xbs[0] = _logged(nc, "pre_a(0)", stage_pre_a, xts.pop(0),
                             True, True)
            xcs[0] = _logged(nc, "pre_b(0)", stage_pre_b, xbs[0], True)
            # steady state: iteration k handles pre_a(k+1), left(k), mid(k),
            # pre_b(k+1), right(k-1), fin(k-1), store(k-1), load(k+3)
            for k in range(BPC + 1):
                if k < BPC:
                    # pre_a(k+1) first: its DVE ops precede mid(k)'s copy in
                    # the in-order DVE queue, so DVE works on pre(k+1) while
                    # PE runs left(k) instead of stalling on mid(k)'s dep.
                    # pre_b(k+1) (waits on Pool's G1-sub) comes after mid(k).
                    if k + 1 < BPC:
                        head = k + 1 <= HEAD_ITEMS
                        xbs[k + 1] = _logged(
                            nc, f"pre_a({k + 1})", stage_pre_a,
                            xts.pop(k + 1), False, head)
                    tts[k] = _logged(nc, f"left({k})", stage_left_mid,
                                     xbs.pop(k), xcs.pop(k))
                    if k + 1 < BPC:
                        xcs[k + 1] = _logged(
                            nc, f"pre_b({k + 1})", stage_pre_b,
                            xbs[k + 1], k + 1 <= HEAD_ITEMS)
                    if k + 3 < BPC:
                        xts[k + 3] = _logged(
                            nc, f"load({k + 3})", stage_load, k + 3)
                if k - 1 >= 0:
                    fine = max(0, (k - 1) - (BPC - 1 - FINE_TAIL))
                    _logged(nc, f"right({k - 1})", stage_right_out,
                            k - 1, tts.pop(k - 1), fine)

    nc.compile()
    return nc


_NC = None
STAGE_LOG = []  # (stage_label, first_inst_idx, last_inst_idx) debug aid


def _logged(nc, label, fn, *args):
    n0 = int(nc.get_next_instruction_name().split("-")[1])
    out = fn(*args)
    n1 = int(nc.get_next_instruction_name().split("-")[1])
    STAGE_LOG.append((label, n0, n1))
    return out


def kernel(x: np.ndarray) -> np.ndarray:
    global _NC
    if _NC is None:
        _NC = _build()
    x = np.ascontiguousarray(
        np.asarray(x, dtype=np.float32).astype(ml_dtypes.bfloat16)
    )
    H = _hadamard(256)
    # hc[p, 0, s, q] = H256[s*128+p, q]; hc[p, 1, s, q] = same / 512
    hrows = H.reshape(2, 128, 256).transpose(1, 0, 2)  # [128, 2, 256]
    hc = np.stack([hrows, hrows / np.float32(512.0)], axis=1)
    hc = np.ascontiguousarray(hc.astype(ml_dtypes.bfloat16))
    xr = x.reshape(NCORES, BPC, 4, 128, D)
    in_maps = [{"x": xr[i], "hc": hc} for i in range(NCORES)]
    res = run_bass_kernel_spmd(_NC, in_maps, list(range(NCORES))).results
    return np.concatenate(
        [np.asarray(r["y"]).reshape(BPC, N, D) for r in res], axis=0
    ).astype(np.float32)
